# revision 1
# baseline (speedup 1.0000x reference)
"""MoE BERT block kernel for 8 Trainium2 NeuronCores.

Strategy: expert parallel. The router (gate matmul + softmax + top-2) is a
~134 MFLOP computation done on the host in float64 while sharding the inputs;
token dispatch by router assignment happens during the host-side shard step.
Each of the 8 cores owns one expert's FFN weights (SBUF-resident, bf16) and
runs the dense FFN over the tokens routed to it (padded to a fixed capacity),
which is >99.9% of the FLOPs. The host then scatter-adds `w * y` per token.

Device math per core (expert e), all tokens column-major (token = free dim):
    H^T = gelu(WupT^T @ X^T + bup)      # [4096, CAP]  bf16, f32 accum
    Y^T = WdownT^T @ H^T + bdown        # [1024, CAP]  f32
"""

import os

os.environ.setdefault("MYCRO_LOCAL_CACHE", "1")

import numpy as np
import ml_dtypes

import concourse.bass as bass
import concourse.bacc as bacc
import concourse.mybir as mybir
import concourse.tile as tile
from concourse.bass_utils import run_bass_kernel_spmd

NUM_EXPERTS = 8
TOP_K = 2
H = 1024
I = 4096
P = 128
CAP = 2161  # per-expert token capacity (= max observed load; mean 2048);
# tokens beyond CAP (never expected for the reference inputs) fall back to a
# host-side numpy computation, so correctness never depends on this margin.
# Uniform tile sizes keep every matmul's streaming time (~180ns at N=432)
# above the ~53ns LDWEIGHTS floor; a small trailing tile would waste it.
TOKEN_TILES = [433, 432, 432, 432, 432]
assert sum(TOKEN_TILES) == CAP

BF16 = mybir.dt.bfloat16
F32 = mybir.dt.float32

_compiled = None  # (nc,) cache — build the Bass program once per process
last_results = None  # BassKernelResults of the most recent run (for profiling)


def _build_program():
    nc = bacc.Bacc("TRN2", target_bir_lowering=False)

    # All inputs arrive pre-permuted into DMA-native per-partition layouts
    # (host packs them), so every transfer has long contiguous lines.
    xt = nc.dram_tensor("xt", [P, (H // P) * CAP], BF16, kind="ExternalInput")
    wup_t = nc.dram_tensor("wup_t", [P, (H // P) * I], BF16, kind="ExternalInput")
    wdn_t = nc.dram_tensor("wdn_t", [P, (I // P) * H], BF16, kind="ExternalInput")
    bup = nc.dram_tensor("bup", [P, I // P], F32, kind="ExternalInput")
    bdn = nc.dram_tensor("bdn", [P, H // P], F32, kind="ExternalInput")
    yt = nc.dram_tensor("yt", [H, CAP], F32, kind="ExternalOutput")

    KO = H // P  # 8 contraction tiles for the up matmul
    IO = I // P  # 32 inter tiles (psum partition tiles up / contraction down)
    HO = H // P  # 8 output tiles for the down matmul

    UPB = 4  # psum banks per up-projection block
    DNB = 4  # psum banks per down-projection block

    with tile.TileContext(nc) as tc:
        with (
            tc.tile_pool(name="weights", bufs=1) as wpool,
            tc.tile_pool(name="xin", bufs=2) as xpool,
            tc.tile_pool(name="hmid", bufs=1) as hpool,
            tc.tile_pool(name="yout", bufs=4) as ypool,
            tc.tile_pool(name="psum_up", bufs=UPB, space="PSUM") as pu,
            tc.tile_pool(name="psum_dn", bufs=DNB, space="PSUM") as pd,
        ):
            yt_r = yt.ap().rearrange("(ho p) t -> p ho t", p=P)
            xt_ap = xt.ap()
            wup_ap = wup_t.ap()
            wdn_ap = wdn_t.ap()

            # DMA issue order is chosen so compute can start early: the first
            # token tile, biases, then the up weights (per-ko chunks, just in
            # time for the ko-outer first block); the down weights stream in
            # per-io chunks interleaved with tile 0's up phase.
            # Up weights split along BOTH ko and io columns, issued in the
            # order tile 0's psum groups consume them (io-group major).  The
            # first column group's chunks are interleaved with x0's per-ko
            # chunks, so group 0's ko-step k is gated on just ~0.4MB; under
            # the 8-core HBM burst the per-step arrival cadence then matches
            # the ko-step compute, and real matmuls start almost immediately.
            UPG = 2 * UPB  # io tiles per tile-0 group
            x0_sb = xpool.tile([P, KO, TOKEN_TILES[0]], BF16, tag="x")
            x0_r = xt_ap[:, 0 : KO * TOKEN_TILES[0]].rearrange(
                "p (ko t) -> p ko t", ko=KO
            )
            wup_sb = wpool.tile([P, KO, I], BF16, tag="wup")
            for ko in range(KO):
                nc.sync.dma_start(x0_sb[:, ko], x0_r[:, ko])
                nc.sync.dma_start(
                    wup_sb[:, ko, 0 : UPG * P],
                    wup_ap[:, ko * I : ko * I + UPG * P],
                )
            bup_sb = wpool.tile([P, IO], F32, tag="bup")
            nc.sync.dma_start(bup_sb[:], bup.ap())
            bdn_sb = wpool.tile([P, HO], F32, tag="bdn")
            nc.sync.dma_start(bdn_sb[:], bdn.ap())
            for iog in range(1, IO // UPG):
                cols = slice(iog * UPG * P, (iog + 1) * UPG * P)
                for ko in range(KO):
                    nc.sync.dma_start(
                        wup_sb[:, ko, cols],
                        wup_ap[:, ko * I + iog * UPG * P : ko * I + (iog + 1) * UPG * P],
                    )
            wdn_sb = wpool.tile([P, IO, H], BF16, tag="wdn")

            # Zeroed tile for warmup / keep-alive matmuls: they have no DMA
            # dependency, so the PE starts immediately and stays busy while
            # weights stream from HBM — keeping the HAM clock gate at full
            # rate.  They accumulate 0*0 = 0 into the first live psum group,
            # which is exact, so no extra psum bank is needed.
            xw_sb = wpool.tile([P, 512], BF16, tag="warmx")
            nc.vector.memset(xw_sb[:], 0.0)

            off = 0
            for t, ntok in enumerate(TOKEN_TILES):
                if t == 0:
                    x_sb = x0_sb
                else:
                    x_sb = xpool.tile([P, KO, TOKEN_TILES[0]], BF16, tag="x")
                    nc.sync.dma_start(
                        x_sb[:, :, :ntok],
                        xt_ap[:, KO * off : KO * (off + ntok)].rearrange(
                            "p (ko t) -> p ko t", ko=KO
                        ),
                    )

                # Up-projection + exact (erf) GELU: H^T tile [4096, ntok].
                # Contraction (ko) outer within a block of psum banks, so a
                # block's matmuls can start as soon as the first weight chunk
                # lands.  Tile 0 uses blocks of 8 banks (the down-projection
                # pool is still idle) with keep-alive matmuls after each
                # ko-step of the first block: its pace is set by the up-weight
                # DMA, and the fillers keep the HAM clock gate from
                # re-throttling during the arrival gaps.
                h_sb = hpool.tile([P, IO, TOKEN_TILES[0]], BF16, tag="h")
                upb = 2 * UPB if t == 0 else UPB
                for blk in range(IO // upb):
                    pss = [
                        (pu if j < UPB else pd).tile(
                            [P, TOKEN_TILES[0]], F32,
                            tag=("pu" if j < UPB else "pd"), name=f"pub{j}",
                        )
                        for j in range(upb)
                    ]
                    warm = t == 0 and blk == 0
                    if warm:
                        # PE warmup before the first data-dependent matmul:
                        # open pss[0]'s accumulation group with zeros.
                        nc.tensor.matmul(
                            pss[0][:, :ntok], lhsT=xw_sb[:, :P],
                            rhs=xw_sb[:, :ntok], start=True, stop=False,
                        )
                        # Bridge the PE to first-chunk arrival (~2-3us) and
                        # start accumulating HAM busy time toward the 3.4us
                        # un-throttle window.
                        for _ in range(6):
                            nc.tensor.matmul(
                                pss[0][:, :ntok], lhsT=xw_sb[:, :P],
                                rhs=xw_sb[:, :ntok], start=False, stop=False,
                            )
                    if t == 0 and blk == IO // upb - 1:
                        # Last tile-0 up block borrows the down pool's psum
                        # banks; close each accumulation group early (j-outer)
                        # so its GELU frees the bank while the rest of the
                        # block computes — otherwise the first down matmul
                        # stalls ~1.5us on the final four GELUs.
                        for j in range(upb):
                            io = blk * upb + j
                            for ko in range(KO):
                                nc.tensor.matmul(
                                    pss[j][:, :ntok],
                                    lhsT=wup_sb[:, ko, io * P : (io + 1) * P],
                                    rhs=x_sb[:, ko, :ntok],
                                    start=(ko == 0),
                                    stop=(ko == KO - 1),
                                )
                    else:
                        for ko in range(KO):
                            for j in range(upb):
                                io = blk * upb + j
                                nc.tensor.matmul(
                                    pss[j][:, :ntok],
                                    lhsT=wup_sb[:, ko, io * P : (io + 1) * P],
                                    rhs=x_sb[:, ko, :ntok],
                                    start=(ko == 0 and not (warm and j == 0)),
                                    stop=(ko == KO - 1),
                                )
                            if warm and ko < KO - 1:
                                # Keep-alive against HBM-contention jitter.
                                nc.tensor.matmul(
                                    pss[0][:, :ntok], lhsT=xw_sb[:, :P],
                                    rhs=xw_sb[:, :ntok], start=False, stop=False,
                                )
                    for j in range(upb):
                        io = blk * upb + j
                        nc.scalar.activation(
                            h_sb[:, io, :ntok],
                            pss[j][:, :ntok],
                            mybir.ActivationFunctionType.Gelu,
                            bias=bup_sb[:, io : io + 1],
                            scale=1.0,
                        )
                    if t == 0:
                        # Stream the down weights while tile 0's up phase runs.
                        for io in range(blk * upb, (blk + 1) * upb):
                            nc.sync.dma_start(
                                wdn_sb[:, io], wdn_ap[:, io * H : (io + 1) * H]
                            )

                # Down-projection + bias: Y^T tile [1024, ntok] f32.
                # Contraction (io) outer within a DNB-bank block: the first
                # matmuls only need h[:, 0] (ready early in the up phase).
                # The very last tile instead runs per-ho contraction sweeps
                # (single bank each) so each output's bias-add + DMA overlaps
                # the next ho's matmuls, shrinking the serial kernel tail.
                if t == len(TOKEN_TILES) - 1:
                    for ho in range(HO):
                        ps1 = pd.tile([P, TOKEN_TILES[0]], F32, tag="pd", name="pdl")
                        for io in range(IO):
                            nc.tensor.matmul(
                                ps1[:, :ntok],
                                lhsT=wdn_sb[:, io, ho * P : (ho + 1) * P],
                                rhs=h_sb[:, io, :ntok],
                                start=(io == 0),
                                stop=(io == IO - 1),
                            )
                        y_sb = ypool.tile([P, TOKEN_TILES[0]], F32, tag="y")
                        nc.vector.tensor_scalar_add(
                            y_sb[:, :ntok], ps1[:, :ntok], bdn_sb[:, ho : ho + 1]
                        )
                        nc.sync.dma_start(
                            yt_r[:, ho, off : off + ntok], y_sb[:, :ntok]
                        )
                else:
                    for blk in range(HO // DNB):
                        ps2 = [pd.tile([P, TOKEN_TILES[0]], F32, tag="pd", name=f"pd{j}") for j in range(DNB)]
                        for io in range(IO):
                            for j in range(DNB):
                                ho = blk * DNB + j
                                nc.tensor.matmul(
                                    ps2[j][:, :ntok],
                                    lhsT=wdn_sb[:, io, ho * P : (ho + 1) * P],
                                    rhs=h_sb[:, io, :ntok],
                                    start=(io == 0),
                                    stop=(io == IO - 1),
                                )
                        for j in range(DNB):
                            ho = blk * DNB + j
                            y_sb = ypool.tile([P, TOKEN_TILES[0]], F32, tag="y")
                            nc.vector.tensor_scalar_add(
                                y_sb[:, :ntok], ps2[j][:, :ntok], bdn_sb[:, ho : ho + 1]
                            )
                            nc.sync.dma_start(
                                yt_r[:, ho, off : off + ntok], y_sb[:, :ntok]
                            )
                off += ntok

    nc.compile()
    return nc


def _get_program():
    global _compiled
    if _compiled is None:
        _compiled = _build_program()
    return _compiled


def _route(X64, Wg64):
    """Replicates the reference router: softmax over gate logits, top-2."""
    T = X64.shape[0]
    logits = X64 @ Wg64.T  # [T, E]
    logits -= logits.max(axis=-1, keepdims=True)
    p = np.exp(logits)
    p /= p.sum(axis=-1, keepdims=True)
    i1 = np.argmax(p, axis=-1)
    rows = np.arange(T)
    w1 = p[rows, i1]
    p2 = p.copy()
    p2[rows, i1] = -1.0
    i2 = np.argmax(p2, axis=-1)
    w2 = p[rows, i2]
    return i1, w1, i2, w2


def kernel(hidden_states, Wg, Wup, bup, Wdown, bdown):
    global last_results
    hidden_states = np.asarray(hidden_states)
    orig_shape = hidden_states.shape
    X = np.ascontiguousarray(hidden_states, dtype=np.float32).reshape(-1, H)
    T = X.shape[0]
    Wg = np.asarray(Wg, dtype=np.float32)
    Wup = np.asarray(Wup, dtype=np.float32)
    bup = np.asarray(bup, dtype=np.float32)
    Wdown = np.asarray(Wdown, dtype=np.float32)
    bdown = np.asarray(bdown, dtype=np.float32)

    # --- Router on host (float64 for a faithful top-2 ordering) ---
    i1, w1, i2, w2 = _route(X.astype(np.float64), Wg.astype(np.float64))

    # --- Dispatch: gather each expert's tokens, pad to CAP ---
    Xb = X.astype(ml_dtypes.bfloat16)
    in_maps = []
    meta = []
    for e in range(NUM_EXPERTS):
        sel1 = np.nonzero(i1 == e)[0]
        sel2 = np.nonzero(i2 == e)[0]
        idx = np.concatenate([sel1, sel2])
        wts = np.concatenate([w1[sel1], w2[sel2]])
        n = idx.size
        overflow = None
        if n > CAP:
            # Never expected for the reference inputs (max load 2161); kept as
            # a correctness safety net: spill tokens are computed on the host.
            overflow = (idx[CAP:], wts[CAP:])
            idx, wts = idx[:CAP], wts[:CAP]
            n = CAP
        idx_pad = np.concatenate([idx, np.zeros(CAP - n, dtype=idx.dtype)])
        # Pack into the kernel's DMA-native per-partition layouts:
        #  xt:  per token tile [P, KO, ntok] blocks, concatenated -> [P, KO*CAP]
        #  wup: Wup.T as [P, KO*I]; wdn: Wdown.T as [P, IO*H]
        xt_full = Xb[idx_pad].T.reshape(H // P, P, CAP)  # [KO, P, CAP]
        blocks = []
        o = 0
        for ntok in TOKEN_TILES:
            blocks.append(xt_full[:, :, o : o + ntok].transpose(1, 0, 2).reshape(P, -1))
            o += ntok
        xt_dev = np.concatenate(blocks, axis=1)
        wup_dev = (
            Wup[e].astype(ml_dtypes.bfloat16).T.reshape(H // P, P, I)
            .transpose(1, 0, 2).reshape(P, -1)
        )
        wdn_dev = (
            Wdown[e].astype(ml_dtypes.bfloat16).T.reshape(I // P, P, H)
            .transpose(1, 0, 2).reshape(P, -1)
        )
        in_maps.append(
            {
                "xt": np.ascontiguousarray(xt_dev),
                "wup_t": np.ascontiguousarray(wup_dev),
                "wdn_t": np.ascontiguousarray(wdn_dev),
                "bup": np.ascontiguousarray(bup[e].reshape(I // P, P).T),
                "bdn": np.ascontiguousarray(bdown[e].reshape(H // P, P).T),
            }
        )
        meta.append((idx, wts, overflow))

    # --- Run the Bass kernel on all 8 cores ---
    nc = _get_program()
    last_results = run_bass_kernel_spmd(nc, in_maps, core_ids=list(range(8)))

    # --- Combine: out[token] += w * y ---
    out = np.zeros((T, H), dtype=np.float32)
    for e in range(NUM_EXPERTS):
        idx, wts, overflow = meta[e]
        yt_full = np.asarray(last_results.results[e]["yt"])  # [H, CAP] f32
        Y = yt_full.T[: idx.size]  # [n, H]
        out[idx] += wts[:, None].astype(np.float32) * Y
        if overflow is not None:
            oidx, owts = overflow
            from scipy.special import erf

            xo = X[oidx]
            h_in = xo @ Wup[e].T + bup[e]
            h = 0.5 * h_in * (1.0 + erf(h_in / np.sqrt(2.0)))
            yo = h @ Wdown[e].T + bdown[e]
            out[oidx] += owts[:, None].astype(np.float32) * yo
    return out.reshape(orig_shape)



# revision 2
# speedup vs baseline: 1.0427x; 1.0427x over previous
"""MoE BERT block kernel for 8 Trainium2 NeuronCores.

Strategy: expert parallel. The router (gate matmul + softmax + top-2) is a
~134 MFLOP computation done on the host in float64 while sharding the inputs;
token dispatch by router assignment happens during the host-side shard step.
Each of the 8 cores owns one expert's FFN weights (SBUF-resident, bf16) and
runs the dense FFN over the tokens routed to it (padded to a fixed capacity),
which is >99.9% of the FLOPs. The host then scatter-adds `w * y` per token.

Device math per core (expert e), all tokens column-major (token = free dim):
    H^T = gelu(WupT^T @ X^T + bup)      # [4096, CAP]  bf16, f32 accum
    Y^T = WdownT^T @ H^T + bdown        # [1024, CAP]  bf16 out

Speed tricks beyond the plain pipelined bf16 GEMMs:
  * Up-projection K-dims 0..255 run as ONE fp8e4 DoubleRow matmul (2x row
    rate, +13%/col) on token tiles 1..4, replacing two bf16 matmuls: ~11%
    faster up phase. fp8 quantization error on a quarter of the contraction
    for 80% of tokens measures 1.57e-2 end-to-end (gate is 2e-2; bf16-only
    is 3.2e-3). The fp8 operands are host-quantized with power-of-two scales
    sx=2^5 (x) and sw=2^11 (Wup); the bf16 Wup copy is host-scaled by
    sx*sw=2^16 so both paths accumulate into the same psum group at the same
    scale, removed for free via the GELU activation's scale=2^-16.
  * Up phase (tiles 1..4) runs ko-INNER per psum bank (8-matmul sweeps), so
    each bank's GELU hides under the next bank's sweep; ko-outer is kept
    only for tile 0 where matmul pace is set by the per-chunk weight DMA.
  * Down phase runs io-inner per-ho sweeps on ALL tiles (the baseline's
    last-tile structure): each ho's bias-add + output DMA hides under the
    next ho's 32-matmul sweep, and no 4-bank group barrier stalls remain.
  * y streams out as bf16 (halves the output DMA), biases re-added there.
"""

import os

os.environ.setdefault("MYCRO_LOCAL_CACHE", "1")

import numpy as np
import ml_dtypes

import concourse.bass as bass
import concourse.bacc as bacc
import concourse.mybir as mybir
import concourse.tile as tile
from concourse.bass_utils import run_bass_kernel_spmd

NUM_EXPERTS = 8
TOP_K = 2
H = 1024
I = 4096
P = 128
CAP = 2161  # per-expert token capacity (= max observed load; mean 2048);
# tokens beyond CAP (never expected for the reference inputs) fall back to a
# host-side numpy computation, so correctness never depends on this margin.
# Uniform tile sizes keep every matmul's streaming time (~180ns at N=432)
# above the ~53ns LDWEIGHTS floor; a small trailing tile would waste it.
TOKEN_TILES = [433, 432, 432, 432, 432]
assert sum(TOKEN_TILES) == CAP

# fp8 up-projection slice: K-dims [0, KF8) are computed by a DoubleRow fp8
# matmul on token tiles 1..4 (tile 0 stays all-bf16: its pace is the weight
# DMA, and skipping it buys error margin). Power-of-two scales; SCALE =
# SX8*SW8 is also premultiplied into the bf16 Wup copy and divided back out
# in the GELU's scale argument, so fp8 and bf16 partials share one psum.
KF8 = 256
SX8 = 2.0**5  # |x| < 5.5 -> |x*sx| < 176 < 224 (TRN2 e4m3 max is 240)
SW8 = 2.0**11  # |wup| < 0.105 -> < 216
SCALE = SX8 * SW8  # 2^16
F8CLIP = 216.0  # rounds to <= 224; keeps host quantization off +-inf

BF16 = mybir.dt.bfloat16
F32 = mybir.dt.float32
F8E4 = mybir.dt.float8e4

_compiled = None  # (nc,) cache — build the Bass program once per process
last_results = None  # BassKernelResults of the most recent run (for profiling)


def _build_program():
    nc = bacc.Bacc("TRN2", target_bir_lowering=False)

    # All inputs arrive pre-permuted into DMA-native per-partition layouts
    # (host packs them), so every transfer has long contiguous lines.
    xt = nc.dram_tensor("xt", [P, (H // P) * CAP], BF16, kind="ExternalInput")
    # fp8 x slice for tiles 1..4: per tile [P, 2, ntok] blocks concatenated.
    xt8 = nc.dram_tensor("xt8", [P, 2 * (CAP - TOKEN_TILES[0])], F8E4, kind="ExternalInput")
    wup_t = nc.dram_tensor("wup_t", [P, (H // P) * I], BF16, kind="ExternalInput")
    # fp8 Wup slice, DoubleRow layout: per io tile [P, 2, 128].
    wup8_t = nc.dram_tensor("wup8_t", [P, (I // P) * 2 * P], F8E4, kind="ExternalInput")
    wdn_t = nc.dram_tensor("wdn_t", [P, (I // P) * H], BF16, kind="ExternalInput")
    bup = nc.dram_tensor("bup", [P, I // P], F32, kind="ExternalInput")
    bdn = nc.dram_tensor("bdn", [P, H // P], F32, kind="ExternalInput")
    yt = nc.dram_tensor("yt", [H, CAP], BF16, kind="ExternalOutput")

    KO = H // P  # 8 contraction tiles for the up matmul
    IO = I // P  # 32 inter tiles (psum partition tiles up / contraction down)
    HO = H // P  # 8 output tiles for the down matmul

    UPB = 4  # psum banks per up-projection block
    DNB = 4  # psum banks used by the down-projection ho sweeps

    GELU_SCALE = 1.0 / SCALE

    with tile.TileContext(nc) as tc:
        with (
            tc.tile_pool(name="weights", bufs=1) as wpool,
            tc.tile_pool(name="xin", bufs=2) as xpool,
            tc.tile_pool(name="hmid", bufs=1) as hpool,
            tc.tile_pool(name="yout", bufs=4) as ypool,
            tc.tile_pool(name="psum_up", bufs=UPB, space="PSUM") as pu,
            tc.tile_pool(name="psum_dn", bufs=DNB, space="PSUM") as pd,
        ):
            yt_r = yt.ap().rearrange("(ho p) t -> p ho t", p=P)
            xt_ap = xt.ap()
            xt8_ap = xt8.ap()
            wup_ap = wup_t.ap()
            wdn_ap = wdn_t.ap()

            # DMA issue order is chosen so compute can start early: the first
            # token tile, biases, then the up weights (per-ko chunks, just in
            # time for the ko-outer first block); the down weights stream in
            # per-io chunks interleaved with tile 0's up phase.
            # Up weights split along BOTH ko and io columns, issued in the
            # order tile 0's psum groups consume them (io-group major).  The
            # first column group's chunks are interleaved with x0's per-ko
            # chunks, so group 0's ko-step k is gated on just ~0.4MB; under
            # the 8-core HBM burst the per-step arrival cadence then matches
            # the ko-step compute, and real matmuls start almost immediately.
            UPG = 2 * UPB  # io tiles per tile-0 group
            x0_sb = xpool.tile([P, KO, TOKEN_TILES[0]], BF16, tag="x")
            x0_r = xt_ap[:, 0 : KO * TOKEN_TILES[0]].rearrange(
                "p (ko t) -> p ko t", ko=KO
            )
            wup_sb = wpool.tile([P, KO, I], BF16, tag="wup")
            for ko in range(KO):
                nc.sync.dma_start(x0_sb[:, ko], x0_r[:, ko])
                nc.sync.dma_start(
                    wup_sb[:, ko, 0 : UPG * P],
                    wup_ap[:, ko * I : ko * I + UPG * P],
                )
            bup_sb = wpool.tile([P, IO], F32, tag="bup")
            nc.sync.dma_start(bup_sb[:], bup.ap())
            bdn_sb = wpool.tile([P, HO], F32, tag="bdn")
            nc.sync.dma_start(bdn_sb[:], bdn.ap())
            for iog in range(1, IO // UPG):
                cols = slice(iog * UPG * P, (iog + 1) * UPG * P)
                for ko in range(KO):
                    nc.sync.dma_start(
                        wup_sb[:, ko, cols],
                        wup_ap[:, ko * I + iog * UPG * P : ko * I + (iog + 1) * UPG * P],
                    )
            wdn_sb = wpool.tile([P, IO, H], BF16, tag="wdn")
            wup8_sb = wpool.tile([P, IO, 2, P], F8E4, tag="wup8")

            # Zeroed tile for warmup / keep-alive matmuls: they have no DMA
            # dependency, so the PE starts immediately and stays busy while
            # weights stream from HBM — keeping the HAM clock gate at full
            # rate.  They accumulate 0*0 = 0 into the first live psum group,
            # which is exact, so no extra psum bank is needed.
            xw_sb = wpool.tile([P, 512], BF16, tag="warmx")
            nc.vector.memset(xw_sb[:], 0.0)

            off = 0
            for t, ntok in enumerate(TOKEN_TILES):
                if t == 0:
                    x_sb = x0_sb
                    x8_sb = None
                else:
                    x_sb = xpool.tile([P, KO, TOKEN_TILES[0]], BF16, tag="x")
                    nc.sync.dma_start(
                        x_sb[:, :, :ntok],
                        xt_ap[:, KO * off : KO * (off + ntok)].rearrange(
                            "p (ko t) -> p ko t", ko=KO
                        ),
                    )
                    # fp8 x pair rows for the DoubleRow matmul; free dim
                    # padded to 448 so the pair-dim stride is 16B-aligned.
                    x8_sb = xpool.tile([P, 2, 448], F8E4, tag="x8")
                    o8 = 2 * (off - TOKEN_TILES[0])
                    nc.sync.dma_start(
                        x8_sb[:, :, :ntok],
                        xt8_ap[:, o8 : o8 + 2 * ntok].rearrange(
                            "p (j t) -> p j t", j=2
                        ),
                    )

                # Up-projection + exact (erf) GELU: H^T tile [4096, ntok].
                h_sb = hpool.tile([P, IO, TOKEN_TILES[0]], BF16, tag="h")
                if t == 0:
                    # Tile 0: contraction (ko) outer within a block of psum
                    # banks, so a block's matmuls can start as soon as the
                    # first weight chunk lands.  Blocks of 8 banks (the down
                    # pool is still idle) with keep-alive matmuls after each
                    # ko-step of the first block: its pace is set by the
                    # up-weight DMA, and the fillers keep the HAM clock gate
                    # from re-throttling during the arrival gaps.
                    upb = 2 * UPB
                    for blk in range(IO // upb):
                        pss = [
                            (pu if j < UPB else pd).tile(
                                [P, TOKEN_TILES[0]], F32,
                                tag=("pu" if j < UPB else "pd"), name=f"pub{j}",
                            )
                            for j in range(upb)
                        ]
                        warm = blk == 0
                        if warm:
                            # PE warmup before the first data-dependent
                            # matmul: open pss[0]'s group with zeros, then
                            # bridge the PE to first-chunk arrival (~2-3us)
                            # while accumulating HAM busy time toward the
                            # 3.4us un-throttle window.
                            nc.tensor.matmul(
                                pss[0][:, :ntok], lhsT=xw_sb[:, :P],
                                rhs=xw_sb[:, :ntok], start=True, stop=False,
                            )
                            for _ in range(9):
                                nc.tensor.matmul(
                                    pss[0][:, :ntok], lhsT=xw_sb[:, :P],
                                    rhs=xw_sb[:, :ntok], start=False, stop=False,
                                )
                        if blk == IO // upb - 1:
                            # Last tile-0 up block borrows the down pool's
                            # psum banks; close each accumulation group early
                            # (j-outer) so its GELU frees the bank while the
                            # rest of the block computes — otherwise the
                            # first down matmul stalls ~1.5us on the final
                            # four GELUs.
                            for j in range(upb):
                                io = blk * upb + j
                                for ko in range(KO):
                                    nc.tensor.matmul(
                                        pss[j][:, :ntok],
                                        lhsT=wup_sb[:, ko, io * P : (io + 1) * P],
                                        rhs=x_sb[:, ko, :ntok],
                                        start=(ko == 0),
                                        stop=(ko == KO - 1),
                                    )
                        else:
                            for ko in range(KO):
                                for j in range(upb):
                                    io = blk * upb + j
                                    nc.tensor.matmul(
                                        pss[j][:, :ntok],
                                        lhsT=wup_sb[:, ko, io * P : (io + 1) * P],
                                        rhs=x_sb[:, ko, :ntok],
                                        start=(ko == 0 and not (warm and j == 0)),
                                        stop=(ko == KO - 1),
                                    )
                                if warm and ko < KO - 1:
                                    # Keep-alive against HBM-contention jitter.
                                    nc.tensor.matmul(
                                        pss[0][:, :ntok], lhsT=xw_sb[:, :P],
                                        rhs=xw_sb[:, :ntok], start=False, stop=False,
                                    )
                        for j in range(upb):
                            io = blk * upb + j
                            nc.scalar.activation(
                                h_sb[:, io, :ntok],
                                pss[j][:, :ntok],
                                mybir.ActivationFunctionType.Gelu,
                                bias=bup_sb[:, io : io + 1],
                                scale=GELU_SCALE,
                            )
                        # Stream the down weights while tile 0's up phase
                        # runs; the fp8 up weights ride at the very end (they
                        # are first needed by tile 1's up phase).
                        for io in range(blk * upb, (blk + 1) * upb):
                            nc.sync.dma_start(
                                wdn_sb[:, io], wdn_ap[:, io * H : (io + 1) * H]
                            )
                        if blk == IO // upb - 1:
                            nc.sync.dma_start(wup8_sb[:], wup8_t.ap())
                else:
                    # Tiles 1..4: weights fully resident, so run ko INNER per
                    # psum bank — one fp8 DoubleRow matmul (K-dims 0..255)
                    # plus six bf16 matmuls per io sweep.  Each bank's GELU
                    # hides under the next bank's 1.3us sweep instead of
                    # stalling a 4-bank group barrier.
                    for io in range(IO):
                        ps = pu.tile([P, TOKEN_TILES[0]], F32, tag="pu", name="pus")
                        nc.tensor.matmul(
                            ps[:, :ntok],
                            lhsT=wup8_sb[:, io],
                            rhs=x8_sb[:, :, :ntok],
                            start=True,
                            stop=False,
                            perf_mode=mybir.MatmulPerfMode.DoubleRow,
                        )
                        for ko in range(KF8 // P, KO):
                            nc.tensor.matmul(
                                ps[:, :ntok],
                                lhsT=wup_sb[:, ko, io * P : (io + 1) * P],
                                rhs=x_sb[:, ko, :ntok],
                                start=False,
                                stop=(ko == KO - 1),
                            )
                        nc.scalar.activation(
                            h_sb[:, io, :ntok],
                            ps[:, :ntok],
                            mybir.ActivationFunctionType.Gelu,
                            bias=bup_sb[:, io : io + 1],
                            scale=GELU_SCALE,
                        )

                # Down-projection + bias: Y^T tile [1024, ntok] bf16 out.
                # io-inner per-ho contraction sweeps (single bank each): each
                # ho's bias-add + output DMA overlaps the next ho's matmuls,
                # so no group barrier and a short serial kernel tail.
                last = t == len(TOKEN_TILES) - 1
                for ho in range(HO):
                    ps1 = pd.tile([P, TOKEN_TILES[0]], F32, tag="pd", name="pdl")
                    for io in range(IO):
                        nc.tensor.matmul(
                            ps1[:, :ntok],
                            lhsT=wdn_sb[:, io, ho * P : (ho + 1) * P],
                            rhs=h_sb[:, io, :ntok],
                            start=(io == 0),
                            stop=(io == IO - 1),
                        )
                    y_sb = ypool.tile([P, TOKEN_TILES[0]], BF16, tag="y")
                    if last and ho == HO - 1:
                        # Split the very last bias+store so the final DMA
                        # starts half a tile earlier.
                        hn = ntok // 2
                        for sl in (slice(0, hn), slice(hn, ntok)):
                            nc.vector.tensor_scalar_add(
                                y_sb[:, sl], ps1[:, sl], bdn_sb[:, ho : ho + 1]
                            )
                            nc.sync.dma_start(
                                yt_r[:, ho, off + sl.start : off + sl.stop],
                                y_sb[:, sl],
                            )
                    else:
                        nc.vector.tensor_scalar_add(
                            y_sb[:, :ntok], ps1[:, :ntok], bdn_sb[:, ho : ho + 1]
                        )
                        nc.sync.dma_start(
                            yt_r[:, ho, off : off + ntok], y_sb[:, :ntok]
                        )
                off += ntok

    nc.compile()
    return nc


def _get_program():
    global _compiled
    if _compiled is None:
        _compiled = _build_program()
    return _compiled


def _route(X64, Wg64):
    """Replicates the reference router: softmax over gate logits, top-2."""
    T = X64.shape[0]
    logits = X64 @ Wg64.T  # [T, E]
    logits -= logits.max(axis=-1, keepdims=True)
    p = np.exp(logits)
    p /= p.sum(axis=-1, keepdims=True)
    i1 = np.argmax(p, axis=-1)
    rows = np.arange(T)
    w1 = p[rows, i1]
    p2 = p.copy()
    p2[rows, i1] = -1.0
    i2 = np.argmax(p2, axis=-1)
    w2 = p[rows, i2]
    return i1, w1, i2, w2


def _q8(a):
    """Host e4m3 quantization (values pre-scaled), saturating, as float32."""
    return np.clip(a, -F8CLIP, F8CLIP).astype(ml_dtypes.float8_e4m3)


def kernel(hidden_states, Wg, Wup, bup, Wdown, bdown):
    global last_results
    hidden_states = np.asarray(hidden_states)
    orig_shape = hidden_states.shape
    X = np.ascontiguousarray(hidden_states, dtype=np.float32).reshape(-1, H)
    T = X.shape[0]
    Wg = np.asarray(Wg, dtype=np.float32)
    Wup = np.asarray(Wup, dtype=np.float32)
    bup = np.asarray(bup, dtype=np.float32)
    Wdown = np.asarray(Wdown, dtype=np.float32)
    bdown = np.asarray(bdown, dtype=np.float32)

    # --- Router on host (float64 for a faithful top-2 ordering) ---
    i1, w1, i2, w2 = _route(X.astype(np.float64), Wg.astype(np.float64))

    # --- Dispatch: gather each expert's tokens, pad to CAP ---
    Xb = X.astype(ml_dtypes.bfloat16)
    T0 = TOKEN_TILES[0]
    in_maps = []
    meta = []
    for e in range(NUM_EXPERTS):
        sel1 = np.nonzero(i1 == e)[0]
        sel2 = np.nonzero(i2 == e)[0]
        idx = np.concatenate([sel1, sel2])
        wts = np.concatenate([w1[sel1], w2[sel2]])
        n = idx.size
        overflow = None
        if n > CAP:
            # Never expected for the reference inputs (max load 2161); kept as
            # a correctness safety net: spill tokens are computed on the host.
            overflow = (idx[CAP:], wts[CAP:])
            idx, wts = idx[:CAP], wts[:CAP]
            n = CAP
        idx_pad = np.concatenate([idx, np.zeros(CAP - n, dtype=idx.dtype)])
        # Pack into the kernel's DMA-native per-partition layouts:
        #  xt:  per token tile [P, KO, ntok] blocks, concatenated -> [P, KO*CAP]
        #  xt8: per token tile [P, 2, ntok] blocks for tiles 1.. (fp8, x*SX8)
        #  wup: Wup.T * SCALE as [P, KO*I]; wup8: [P, IO, 2, 128] (fp8, *SW8)
        #  wdn: Wdown.T as [P, IO*H]
        xt_full = Xb[idx_pad].T.reshape(H // P, P, CAP)  # [KO, P, CAP]
        x8_full = _q8(X[idx_pad, :KF8] * SX8).reshape(CAP, 2, P)  # [CAP, 2, P]
        blocks = []
        blocks8 = []
        o = 0
        for ti, ntok in enumerate(TOKEN_TILES):
            blocks.append(xt_full[:, :, o : o + ntok].transpose(1, 0, 2).reshape(P, -1))
            if ti > 0:
                blocks8.append(
                    x8_full[o : o + ntok].transpose(2, 1, 0).reshape(P, -1)
                )
            o += ntok
        xt_dev = np.concatenate(blocks, axis=1)
        xt8_dev = np.concatenate(blocks8, axis=1)
        wup_dev = (
            (Wup[e] * SCALE).astype(ml_dtypes.bfloat16).T.reshape(H // P, P, I)
            .transpose(1, 0, 2).reshape(P, -1)
        )
        # wup8[p, io, j, m] = Wup[io*128+m, 128*j+p] * SW8
        wup8_dev = (
            _q8(Wup[e][:, :KF8] * SW8)
            .reshape(I // P, P, 2, P)  # [io, m, j, p]
            .transpose(3, 0, 2, 1)
            .reshape(P, -1)
        )
        wdn_dev = (
            Wdown[e].astype(ml_dtypes.bfloat16).T.reshape(I // P, P, H)
            .transpose(1, 0, 2).reshape(P, -1)
        )
        in_maps.append(
            {
                "xt": np.ascontiguousarray(xt_dev),
                "xt8": np.ascontiguousarray(xt8_dev),
                "wup_t": np.ascontiguousarray(wup_dev),
                "wup8_t": np.ascontiguousarray(wup8_dev),
                "wdn_t": np.ascontiguousarray(wdn_dev),
                "bup": np.ascontiguousarray(bup[e].reshape(I // P, P).T),
                "bdn": np.ascontiguousarray(bdown[e].reshape(H // P, P).T),
            }
        )
        meta.append((idx, wts, overflow))

    # --- Run the Bass kernel on all 8 cores ---
    nc = _get_program()
    last_results = run_bass_kernel_spmd(nc, in_maps, core_ids=list(range(8)))

    # --- Combine: out[token] += w * y ---
    out = np.zeros((T, H), dtype=np.float32)
    for e in range(NUM_EXPERTS):
        idx, wts, overflow = meta[e]
        yt_full = np.asarray(last_results.results[e]["yt"])  # [H, CAP] bf16
        Y = yt_full.T[: idx.size].astype(np.float32)  # [n, H]
        out[idx] += wts[:, None].astype(np.float32) * Y
        if overflow is not None:
            oidx, owts = overflow
            from scipy.special import erf

            xo = X[oidx]
            h_in = xo @ Wup[e].T + bup[e]
            h = 0.5 * h_in * (1.0 + erf(h_in / np.sqrt(2.0)))
            yo = h @ Wdown[e].T + bdown[e]
            out[oidx] += owts[:, None].astype(np.float32) * yo
    return out.reshape(orig_shape)


# revision 4
# speedup vs baseline: 1.0433x; 1.0006x over previous
"""MoE BERT block kernel for 8 Trainium2 NeuronCores.

Strategy: expert parallel. The router (gate matmul + softmax + top-2) is a
~134 MFLOP computation done on the host in float64 while sharding the inputs;
token dispatch by router assignment happens during the host-side shard step.
Each of the 8 cores owns one expert's FFN weights (SBUF-resident, bf16) and
runs the dense FFN over the tokens routed to it (padded to a fixed capacity),
which is >99.9% of the FLOPs. The host then scatter-adds `w * y` per token.

Device math per core (expert e), all tokens column-major (token = free dim):
    H^T = gelu(WupT^T @ X^T + bup)      # [4096, CAP]  bf16, f32 accum
    Y^T = WdownT^T @ H^T + bdown        # [1024, CAP]  bf16 out

Speed tricks beyond the plain pipelined bf16 GEMMs:
  * Up-projection K-dims 0..255 run as ONE fp8e4 DoubleRow matmul (2x row
    rate, +13%/col) on token tiles 1..4, replacing two bf16 matmuls: ~11%
    faster up phase. fp8 quantization error on a quarter of the contraction
    for 80% of tokens measures 1.57e-2 end-to-end (gate is 2e-2; bf16-only
    is 3.2e-3). The fp8 operands are host-quantized with power-of-two scales
    sx=2^5 (x) and sw=2^11 (Wup); the bf16 Wup copy is host-scaled by
    sx*sw=2^16 so both paths accumulate into the same psum group at the same
    scale, removed for free via the GELU activation's scale=2^-16.
  * Up phase (tiles 1..4) runs ko-INNER per psum bank (8-matmul sweeps), so
    each bank's GELU hides under the next bank's sweep; ko-outer is kept
    only for tile 0 where matmul pace is set by the per-chunk weight DMA.
  * Down phase runs io-inner per-ho sweeps on ALL tiles (the baseline's
    last-tile structure): each ho's bias-add + output DMA hides under the
    next ho's 32-matmul sweep, and no 4-bank group barrier stalls remain.
  * y streams out as bf16 (halves the output DMA), biases re-added there.
"""

import os

os.environ.setdefault("MYCRO_LOCAL_CACHE", "1")

import numpy as np
import ml_dtypes

import concourse.bass as bass
import concourse.bacc as bacc
import concourse.mybir as mybir
import concourse.tile as tile
from concourse.bass_utils import run_bass_kernel_spmd

NUM_EXPERTS = 8
TOP_K = 2
H = 1024
I = 4096
P = 128
CAP = 2161  # per-expert token capacity (= max observed load; mean 2048);
# tokens beyond CAP (never expected for the reference inputs) fall back to a
# host-side numpy computation, so correctness never depends on this margin.
# Uniform tile sizes keep every matmul's streaming time (~180ns at N=432)
# above the ~53ns LDWEIGHTS floor; a small trailing tile would waste it.
TOKEN_TILES = [433, 432, 432, 432, 432]
assert sum(TOKEN_TILES) == CAP

# fp8 up-projection slice: K-dims [0, KF8) are computed by a DoubleRow fp8
# matmul on token tiles 1..4 (tile 0 stays all-bf16: its pace is the weight
# DMA, and skipping it buys error margin). Power-of-two scales; SCALE =
# SX8*SW8 is also premultiplied into the bf16 Wup copy and divided back out
# in the GELU's scale argument, so fp8 and bf16 partials share one psum.
KF8 = 256
SX8 = 2.0**5  # |x| < 5.5 -> |x*sx| < 176 < 224 (TRN2 e4m3 max is 240)
SW8 = 2.0**11  # |wup| < 0.105 -> < 216
SCALE = SX8 * SW8  # 2^16
F8CLIP = 216.0  # rounds to <= 224; keeps host quantization off +-inf

BF16 = mybir.dt.bfloat16
F32 = mybir.dt.float32
F8E4 = mybir.dt.float8e4

_compiled = None  # (nc,) cache — build the Bass program once per process
last_results = None  # BassKernelResults of the most recent run (for profiling)


def _build_program():
    nc = bacc.Bacc("TRN2", target_bir_lowering=False)

    # All inputs arrive pre-permuted into DMA-native per-partition layouts
    # (host packs them), so every transfer has long contiguous lines.
    xt = nc.dram_tensor("xt", [P, (H // P) * CAP], BF16, kind="ExternalInput")
    # fp8 x slice for tiles 1..4: per tile [P, 2, ntok] blocks concatenated.
    xt8 = nc.dram_tensor("xt8", [P, 2 * (CAP - TOKEN_TILES[0])], F8E4, kind="ExternalInput")
    wup_t = nc.dram_tensor("wup_t", [P, (H // P) * I], BF16, kind="ExternalInput")
    # fp8 Wup slice, DoubleRow layout: per io tile [P, 2, 128].
    wup8_t = nc.dram_tensor("wup8_t", [P, (I // P) * 2 * P], F8E4, kind="ExternalInput")
    wdn_t = nc.dram_tensor("wdn_t", [P, (I // P) * H], BF16, kind="ExternalInput")
    bup = nc.dram_tensor("bup", [P, I // P], F32, kind="ExternalInput")
    bdn = nc.dram_tensor("bdn", [P, H // P], F32, kind="ExternalInput")
    yt = nc.dram_tensor("yt", [H, CAP], BF16, kind="ExternalOutput")

    KO = H // P  # 8 contraction tiles for the up matmul
    IO = I // P  # 32 inter tiles (psum partition tiles up / contraction down)
    HO = H // P  # 8 output tiles for the down matmul

    UPB = 4  # psum banks per up-projection block
    DNB = 4  # psum banks used by the down-projection ho sweeps

    GELU_SCALE = 1.0 / SCALE

    with tile.TileContext(nc) as tc:
        with (
            tc.tile_pool(name="weights", bufs=1) as wpool,
            tc.tile_pool(name="xin", bufs=2) as xpool,
            tc.tile_pool(name="hmid", bufs=1) as hpool,
            tc.tile_pool(name="yout", bufs=4) as ypool,
            tc.tile_pool(name="psum_up", bufs=UPB, space="PSUM") as pu,
            tc.tile_pool(name="psum_dn", bufs=DNB, space="PSUM") as pd,
        ):
            yt_r = yt.ap().rearrange("(ho p) t -> p ho t", p=P)
            xt_ap = xt.ap()
            xt8_ap = xt8.ap()
            wup_ap = wup_t.ap()
            wdn_ap = wdn_t.ap()

            # DMA issue order is chosen so compute can start early: the first
            # token tile, biases, then the up weights (per-ko chunks, just in
            # time for the ko-outer first block); the down weights stream in
            # per-io chunks interleaved with tile 0's up phase.
            # Up weights split along BOTH ko and io columns, issued in the
            # order tile 0's psum groups consume them (io-group major).  The
            # first column group's chunks are interleaved with x0's per-ko
            # chunks, so group 0's ko-step k is gated on just ~0.4MB; under
            # the 8-core HBM burst the per-step arrival cadence then matches
            # the ko-step compute, and real matmuls start almost immediately.
            UPG = 2 * UPB  # io tiles per tile-0 group
            x0_sb = xpool.tile([P, KO, TOKEN_TILES[0]], BF16, tag="x")
            x0_r = xt_ap[:, 0 : KO * TOKEN_TILES[0]].rearrange(
                "p (ko t) -> p ko t", ko=KO
            )
            wup_sb = wpool.tile([P, KO, I], BF16, tag="wup")
            for ko in range(KO):
                nc.sync.dma_start(x0_sb[:, ko], x0_r[:, ko])
                nc.sync.dma_start(
                    wup_sb[:, ko, 0 : UPG * P],
                    wup_ap[:, ko * I : ko * I + UPG * P],
                )
            bup_sb = wpool.tile([P, IO], F32, tag="bup")
            nc.sync.dma_start(bup_sb[:], bup.ap())
            bdn_sb = wpool.tile([P, HO], F32, tag="bdn")
            nc.sync.dma_start(bdn_sb[:], bdn.ap())
            for iog in range(1, IO // UPG):
                cols = slice(iog * UPG * P, (iog + 1) * UPG * P)
                for ko in range(KO):
                    nc.sync.dma_start(
                        wup_sb[:, ko, cols],
                        wup_ap[:, ko * I + iog * UPG * P : ko * I + (iog + 1) * UPG * P],
                    )
            wdn_sb = wpool.tile([P, IO, H], BF16, tag="wdn")
            wup8_sb = wpool.tile([P, IO, 2, P], F8E4, tag="wup8")

            # Zeroed tile for warmup / keep-alive matmuls: they have no DMA
            # dependency, so the PE starts immediately and stays busy while
            # weights stream from HBM — keeping the HAM clock gate at full
            # rate.  They accumulate 0*0 = 0 into the first live psum group,
            # which is exact, so no extra psum bank is needed.
            xw_sb = wpool.tile([P, 512], BF16, tag="warmx")
            nc.vector.memset(xw_sb[:], 0.0)

            off = 0
            for t, ntok in enumerate(TOKEN_TILES):
                if t == 0:
                    x_sb = x0_sb
                    x8_sb = None
                else:
                    x_sb = xpool.tile([P, KO, TOKEN_TILES[0]], BF16, tag="x")
                    nc.sync.dma_start(
                        x_sb[:, :, :ntok],
                        xt_ap[:, KO * off : KO * (off + ntok)].rearrange(
                            "p (ko t) -> p ko t", ko=KO
                        ),
                    )
                    # fp8 x pair rows for the DoubleRow matmul; free dim
                    # padded to 448 so the pair-dim stride is 16B-aligned.
                    x8_sb = xpool.tile([P, 2, 448], F8E4, tag="x8")
                    o8 = 2 * (off - TOKEN_TILES[0])
                    nc.sync.dma_start(
                        x8_sb[:, :, :ntok],
                        xt8_ap[:, o8 : o8 + 2 * ntok].rearrange(
                            "p (j t) -> p j t", j=2
                        ),
                    )

                # Up-projection + exact (erf) GELU: H^T tile [4096, ntok].
                h_sb = hpool.tile([P, IO, TOKEN_TILES[0]], BF16, tag="h")
                if t == 0:
                    # Tile 0: contraction (ko) outer within a block of psum
                    # banks, so a block's matmuls can start as soon as the
                    # first weight chunk lands.  Blocks of 8 banks (the down
                    # pool is still idle) with keep-alive matmuls after each
                    # ko-step of the first block: its pace is set by the
                    # up-weight DMA, and the fillers keep the HAM clock gate
                    # from re-throttling during the arrival gaps.
                    upb = 2 * UPB
                    for blk in range(IO // upb):
                        pss = [
                            (pu if j < UPB else pd).tile(
                                [P, TOKEN_TILES[0]], F32,
                                tag=("pu" if j < UPB else "pd"), name=f"pub{j}",
                            )
                            for j in range(upb)
                        ]
                        warm = blk == 0
                        if warm:
                            # PE warmup before the first data-dependent
                            # matmul: open pss[0]'s group with zeros, then
                            # bridge the PE to first-chunk arrival (~2-3us)
                            # while accumulating HAM busy time toward the
                            # 3.4us un-throttle window.
                            nc.tensor.matmul(
                                pss[0][:, :ntok], lhsT=xw_sb[:, :P],
                                rhs=xw_sb[:, :ntok], start=True, stop=False,
                            )
                            for _ in range(9):
                                nc.tensor.matmul(
                                    pss[0][:, :ntok], lhsT=xw_sb[:, :P],
                                    rhs=xw_sb[:, :ntok], start=False, stop=False,
                                )
                        if blk == IO // upb - 1:
                            # Last tile-0 up block borrows the down pool's
                            # psum banks; close each accumulation group early
                            # (j-outer) so its GELU frees the bank while the
                            # rest of the block computes — otherwise the
                            # first down matmul stalls ~1.5us on the final
                            # four GELUs.
                            for j in range(upb):
                                io = blk * upb + j
                                for ko in range(KO):
                                    nc.tensor.matmul(
                                        pss[j][:, :ntok],
                                        lhsT=wup_sb[:, ko, io * P : (io + 1) * P],
                                        rhs=x_sb[:, ko, :ntok],
                                        start=(ko == 0),
                                        stop=(ko == KO - 1),
                                    )
                        else:
                            for ko in range(KO):
                                for j in range(upb):
                                    io = blk * upb + j
                                    nc.tensor.matmul(
                                        pss[j][:, :ntok],
                                        lhsT=wup_sb[:, ko, io * P : (io + 1) * P],
                                        rhs=x_sb[:, ko, :ntok],
                                        start=(ko == 0 and not (warm and j == 0)),
                                        stop=(ko == KO - 1),
                                    )
                                if warm and ko < KO - 1:
                                    # Keep-alive against HBM-contention jitter.
                                    nc.tensor.matmul(
                                        pss[0][:, :ntok], lhsT=xw_sb[:, :P],
                                        rhs=xw_sb[:, :ntok], start=False, stop=False,
                                    )
                        for j in range(upb):
                            io = blk * upb + j
                            nc.scalar.activation(
                                h_sb[:, io, :ntok],
                                pss[j][:, :ntok],
                                mybir.ActivationFunctionType.Gelu,
                                bias=bup_sb[:, io : io + 1],
                                scale=GELU_SCALE,
                            )
                        # Stream the down weights while tile 0's up phase
                        # runs; the fp8 up weights ride at the very end (they
                        # are first needed by tile 1's up phase).
                        for io in range(blk * upb, (blk + 1) * upb):
                            nc.sync.dma_start(
                                wdn_sb[:, io], wdn_ap[:, io * H : (io + 1) * H]
                            )
                        if blk == IO // upb - 1:
                            nc.sync.dma_start(wup8_sb[:], wup8_t.ap())
                else:
                    # Tiles 1..4: weights fully resident, so run ko INNER in
                    # io PAIRS alternating between two psum banks — one fp8
                    # DoubleRow matmul (K-dims 0..255) plus six bf16 matmuls
                    # per io.  Alternating banks avoids the ~2ns/matmul
                    # same-bank accumulation penalty, and each pair's GELUs
                    # hide under the next pair's 2.6us sweep instead of
                    # stalling a 4-bank group barrier.
                    for iop in range(IO // 2):
                        pspair = [
                            pu.tile([P, TOKEN_TILES[0]], F32, tag="pu", name=f"pus{j}")
                            for j in range(2)
                        ]
                        for j in range(2):
                            nc.tensor.matmul(
                                pspair[j][:, :ntok],
                                lhsT=wup8_sb[:, 2 * iop + j],
                                rhs=x8_sb[:, :, :ntok],
                                start=True,
                                stop=False,
                                perf_mode=mybir.MatmulPerfMode.DoubleRow,
                            )
                        for ko in range(KF8 // P, KO):
                            for j in range(2):
                                io = 2 * iop + j
                                nc.tensor.matmul(
                                    pspair[j][:, :ntok],
                                    lhsT=wup_sb[:, ko, io * P : (io + 1) * P],
                                    rhs=x_sb[:, ko, :ntok],
                                    start=False,
                                    stop=(ko == KO - 1),
                                )
                        for j in range(2):
                            io = 2 * iop + j
                            nc.scalar.activation(
                                h_sb[:, io, :ntok],
                                pspair[j][:, :ntok],
                                mybir.ActivationFunctionType.Gelu,
                                bias=bup_sb[:, io : io + 1],
                                scale=GELU_SCALE,
                            )

                # Down-projection + bias: Y^T tile [1024, ntok] bf16 out.
                last = t == len(TOKEN_TILES) - 1
                if t == 0:
                    # Tile 0: contraction (io) outer within a DNB-bank block:
                    # the io demand is spread across the whole phase, matching
                    # the down-weight chunks still arriving from HBM.
                    for blk in range(HO // DNB):
                        ps2 = [
                            pd.tile([P, TOKEN_TILES[0]], F32, tag="pd", name=f"pd{j}")
                            for j in range(DNB)
                        ]
                        for io in range(IO):
                            for j in range(DNB):
                                ho = blk * DNB + j
                                nc.tensor.matmul(
                                    ps2[j][:, :ntok],
                                    lhsT=wdn_sb[:, io, ho * P : (ho + 1) * P],
                                    rhs=h_sb[:, io, :ntok],
                                    start=(io == 0),
                                    stop=(io == IO - 1),
                                )
                        for j in range(DNB):
                            ho = blk * DNB + j
                            y_sb = ypool.tile([P, TOKEN_TILES[0]], BF16, tag="y")
                            nc.vector.tensor_scalar_add(
                                y_sb[:, :ntok], ps2[j][:, :ntok], bdn_sb[:, ho : ho + 1]
                            )
                            nc.sync.dma_start(
                                yt_r[:, ho, off : off + ntok], y_sb[:, :ntok]
                            )
                else:
                    # Tiles 1..4: io-inner contraction sweeps over ho PAIRS
                    # alternating two psum banks: no same-bank accumulation
                    # penalty, and each pair's bias-add + output DMA overlaps
                    # the next pair's 11.8us sweep — no group barrier and a
                    # short serial kernel tail.
                    for hop in range(HO // 2):
                        pspair = [
                            pd.tile([P, TOKEN_TILES[0]], F32, tag="pd", name=f"pdl{j}")
                            for j in range(2)
                        ]
                        for io in range(IO):
                            for j in range(2):
                                ho = 2 * hop + j
                                nc.tensor.matmul(
                                    pspair[j][:, :ntok],
                                    lhsT=wdn_sb[:, io, ho * P : (ho + 1) * P],
                                    rhs=h_sb[:, io, :ntok],
                                    start=(io == 0),
                                    stop=(io == IO - 1),
                                )
                        for j in range(2):
                            ho = 2 * hop + j
                            y_sb = ypool.tile([P, TOKEN_TILES[0]], BF16, tag="y")
                            if last and ho == HO - 1:
                                # Split the very last bias+store so the final
                                # DMA starts half a tile earlier.
                                hn = ntok // 2
                                for sl in (slice(0, hn), slice(hn, ntok)):
                                    nc.vector.tensor_scalar_add(
                                        y_sb[:, sl], pspair[j][:, sl],
                                        bdn_sb[:, ho : ho + 1],
                                    )
                                    nc.sync.dma_start(
                                        yt_r[:, ho, off + sl.start : off + sl.stop],
                                        y_sb[:, sl],
                                    )
                            else:
                                nc.vector.tensor_scalar_add(
                                    y_sb[:, :ntok], pspair[j][:, :ntok],
                                    bdn_sb[:, ho : ho + 1],
                                )
                                nc.sync.dma_start(
                                    yt_r[:, ho, off : off + ntok], y_sb[:, :ntok]
                                )
                off += ntok

    nc.compile()
    return nc


def _get_program():
    global _compiled
    if _compiled is None:
        _compiled = _build_program()
    return _compiled


def _route(X64, Wg64):
    """Replicates the reference router: softmax over gate logits, top-2."""
    T = X64.shape[0]
    logits = X64 @ Wg64.T  # [T, E]
    logits -= logits.max(axis=-1, keepdims=True)
    p = np.exp(logits)
    p /= p.sum(axis=-1, keepdims=True)
    i1 = np.argmax(p, axis=-1)
    rows = np.arange(T)
    w1 = p[rows, i1]
    p2 = p.copy()
    p2[rows, i1] = -1.0
    i2 = np.argmax(p2, axis=-1)
    w2 = p[rows, i2]
    return i1, w1, i2, w2


def _q8(a):
    """Host e4m3 quantization (values pre-scaled), saturating, as float32."""
    return np.clip(a, -F8CLIP, F8CLIP).astype(ml_dtypes.float8_e4m3)


def kernel(hidden_states, Wg, Wup, bup, Wdown, bdown):
    global last_results
    hidden_states = np.asarray(hidden_states)
    orig_shape = hidden_states.shape
    X = np.ascontiguousarray(hidden_states, dtype=np.float32).reshape(-1, H)
    T = X.shape[0]
    Wg = np.asarray(Wg, dtype=np.float32)
    Wup = np.asarray(Wup, dtype=np.float32)
    bup = np.asarray(bup, dtype=np.float32)
    Wdown = np.asarray(Wdown, dtype=np.float32)
    bdown = np.asarray(bdown, dtype=np.float32)

    # --- Router on host (float64 for a faithful top-2 ordering) ---
    i1, w1, i2, w2 = _route(X.astype(np.float64), Wg.astype(np.float64))

    # --- Dispatch: gather each expert's tokens, pad to CAP ---
    Xb = X.astype(ml_dtypes.bfloat16)
    T0 = TOKEN_TILES[0]
    in_maps = []
    meta = []
    for e in range(NUM_EXPERTS):
        sel1 = np.nonzero(i1 == e)[0]
        sel2 = np.nonzero(i2 == e)[0]
        idx = np.concatenate([sel1, sel2])
        wts = np.concatenate([w1[sel1], w2[sel2]])
        n = idx.size
        overflow = None
        if n > CAP:
            # Never expected for the reference inputs (max load 2161); kept as
            # a correctness safety net: spill tokens are computed on the host.
            overflow = (idx[CAP:], wts[CAP:])
            idx, wts = idx[:CAP], wts[:CAP]
            n = CAP
        idx_pad = np.concatenate([idx, np.zeros(CAP - n, dtype=idx.dtype)])
        # Pack into the kernel's DMA-native per-partition layouts:
        #  xt:  per token tile [P, KO, ntok] blocks, concatenated -> [P, KO*CAP]
        #  xt8: per token tile [P, 2, ntok] blocks for tiles 1.. (fp8, x*SX8)
        #  wup: Wup.T * SCALE as [P, KO*I]; wup8: [P, IO, 2, 128] (fp8, *SW8)
        #  wdn: Wdown.T as [P, IO*H]
        xt_full = Xb[idx_pad].T.reshape(H // P, P, CAP)  # [KO, P, CAP]
        x8_full = _q8(X[idx_pad, :KF8] * SX8).reshape(CAP, 2, P)  # [CAP, 2, P]
        blocks = []
        blocks8 = []
        o = 0
        for ti, ntok in enumerate(TOKEN_TILES):
            blocks.append(xt_full[:, :, o : o + ntok].transpose(1, 0, 2).reshape(P, -1))
            if ti > 0:
                blocks8.append(
                    x8_full[o : o + ntok].transpose(2, 1, 0).reshape(P, -1)
                )
            o += ntok
        xt_dev = np.concatenate(blocks, axis=1)
        xt8_dev = np.concatenate(blocks8, axis=1)
        wup_dev = (
            (Wup[e] * SCALE).astype(ml_dtypes.bfloat16).T.reshape(H // P, P, I)
            .transpose(1, 0, 2).reshape(P, -1)
        )
        # wup8[p, io, j, m] = Wup[io*128+m, 128*j+p] * SW8
        wup8_dev = (
            _q8(Wup[e][:, :KF8] * SW8)
            .reshape(I // P, P, 2, P)  # [io, m, j, p]
            .transpose(3, 0, 2, 1)
            .reshape(P, -1)
        )
        wdn_dev = (
            Wdown[e].astype(ml_dtypes.bfloat16).T.reshape(I // P, P, H)
            .transpose(1, 0, 2).reshape(P, -1)
        )
        in_maps.append(
            {
                "xt": np.ascontiguousarray(xt_dev),
                "xt8": np.ascontiguousarray(xt8_dev),
                "wup_t": np.ascontiguousarray(wup_dev),
                "wup8_t": np.ascontiguousarray(wup8_dev),
                "wdn_t": np.ascontiguousarray(wdn_dev),
                "bup": np.ascontiguousarray(bup[e].reshape(I // P, P).T),
                "bdn": np.ascontiguousarray(bdown[e].reshape(H // P, P).T),
            }
        )
        meta.append((idx, wts, overflow))

    # --- Run the Bass kernel on all 8 cores ---
    nc = _get_program()
    last_results = run_bass_kernel_spmd(nc, in_maps, core_ids=list(range(8)))

    # --- Combine: out[token] += w * y ---
    out = np.zeros((T, H), dtype=np.float32)
    for e in range(NUM_EXPERTS):
        idx, wts, overflow = meta[e]
        yt_full = np.asarray(last_results.results[e]["yt"])  # [H, CAP] bf16
        Y = yt_full.T[: idx.size].astype(np.float32)  # [n, H]
        out[idx] += wts[:, None].astype(np.float32) * Y
        if overflow is not None:
            oidx, owts = overflow
            from scipy.special import erf

            xo = X[oidx]
            h_in = xo @ Wup[e].T + bup[e]
            h = 0.5 * h_in * (1.0 + erf(h_in / np.sqrt(2.0)))
            yo = h @ Wdown[e].T + bdown[e]
            out[oidx] += owts[:, None].astype(np.float32) * yo
    return out.reshape(orig_shape)


# revision 5
# speedup vs baseline: 1.0625x; 1.0184x over previous
"""MoE BERT block kernel for 8 Trainium2 NeuronCores.

Strategy: expert parallel. The router (gate matmul + softmax + top-2) is a
~134 MFLOP computation done on the host in float64 while sharding the inputs;
token dispatch by router assignment happens during the host-side shard step.
Each of the 8 cores owns one expert's FFN weights (SBUF-resident) and runs
the dense FFN over the tokens routed to it (padded to a fixed capacity),
which is >99.9% of the FLOPs. The host then scatter-adds `w * y` per token.

Device math per core (expert e), all tokens column-major (token = free dim):
    H^T = gelu(WupT^T @ X^T + bup)      # [4096, CAP]  bf16, f32 accum
    Y^T = WdownT^T @ H^T + bdown        # [1024, CAP]  bf16 out

Speed tricks beyond the plain pipelined bf16 GEMMs:
  * Up-projection K-dims 0..255 run as ONE fp8e4 DoubleRow matmul (2x row
    rate, +13%/col) instead of two bf16 matmuls: ~11% faster up phase, and
    during tile 0 (paced by the weight DMA under the 8-core HBM burst) the
    fp8 slice also halves those K-dims' weight bytes.  fp8 quantization on a
    quarter of the up contraction measures 1.73e-2 end-to-end (gate is 2e-2;
    all-bf16 is 3.2e-3).  The fp8 operands are host-quantized with
    power-of-two scales sx=2^5 (x) and sw=2^11 (Wup); the bf16 Wup copy is
    host-scaled by sx*sw=2^16 so both paths accumulate into the same psum
    group at the same scale, removed for free via the GELU's scale=2^-16.
  * Up phase (tiles 1..4) runs ko INNER over io PAIRS alternating two psum
    banks: no ~2ns/matmul same-bank accumulation penalty, GELUs hide under
    the next pair's sweep, no 4-bank group barrier.  Tile 0 keeps ko-outer
    blocks so its matmul order matches the per-chunk weight arrival.
  * Down phase (tiles 1..4) runs io-inner sweeps over ho PAIRS (same idea);
    each pair's bias-add + output DMA hides under the next pair's sweep.
    Tile 0 keeps io-outer blocks, matching the still-arriving down weights.
  * y streams out as bf16 (halves the output DMA), biases re-added there.
"""

import os

os.environ.setdefault("MYCRO_LOCAL_CACHE", "1")

import numpy as np
import ml_dtypes

import concourse.bass as bass
import concourse.bacc as bacc
import concourse.mybir as mybir
import concourse.tile as tile
from concourse.bass_utils import run_bass_kernel_spmd

NUM_EXPERTS = 8
TOP_K = 2
H = 1024
I = 4096
P = 128
CAP = 2161  # per-expert token capacity (= max observed load; mean 2048);
# tokens beyond CAP (never expected for the reference inputs) fall back to a
# host-side numpy computation, so correctness never depends on this margin.
# Uniform tile sizes keep every matmul's streaming time (~180ns at N=432)
# above the ~53ns LDWEIGHTS floor; a small trailing tile would waste it.
TOKEN_TILES = [433, 432, 432, 432, 432]
assert sum(TOKEN_TILES) == CAP

# fp8 up-projection slice: K-dims [0, KF8) are computed by a DoubleRow fp8
# matmul. Power-of-two scales; SCALE = SX8*SW8 is also premultiplied into
# the bf16 Wup copy and divided back out in the GELU's scale argument, so
# fp8 and bf16 partials share one psum accumulation group.
KF8 = 256
KO8 = KF8 // P  # 2 bf16 ko-chunks replaced by the fp8 DoubleRow matmul
SX8 = 2.0**5  # |x| < 5.5 -> |x*sx| < 176 < 224 (TRN2 e4m3 max is 240)
SW8 = 2.0**11  # |wup| < 0.105 -> < 216
SCALE = SX8 * SW8  # 2^16
F8CLIP = 216.0  # rounds to <= 224; keeps host quantization off +-inf

BF16 = mybir.dt.bfloat16
F32 = mybir.dt.float32
F8E4 = mybir.dt.float8e4

_compiled = None  # (nc,) cache — build the Bass program once per process
last_results = None  # BassKernelResults of the most recent run (for profiling)


def _build_program():
    nc = bacc.Bacc("TRN2", target_bir_lowering=False)

    KO = H // P  # 8 contraction tiles for the up matmul (2 fp8 + 6 bf16)
    KB = KO - KO8  # bf16 ko-chunks (ko 2..7)
    IO = I // P  # 32 inter tiles (psum partition tiles up / contraction down)
    HO = H // P  # 8 output tiles for the down matmul

    # All inputs arrive pre-permuted into DMA-native per-partition layouts
    # (host packs them), so every transfer has long contiguous lines.
    xt = nc.dram_tensor("xt", [P, KB * CAP], BF16, kind="ExternalInput")
    # fp8 x pair rows (K-dims 0..255): per tile [P, 2, ntok] blocks.
    xt8 = nc.dram_tensor("xt8", [P, 2 * CAP], F8E4, kind="ExternalInput")
    wup_t = nc.dram_tensor("wup_t", [P, KB * I], BF16, kind="ExternalInput")
    # fp8 Wup slice, DoubleRow layout: per io tile [P, 2, 128].
    wup8_t = nc.dram_tensor("wup8_t", [P, IO * 2 * P], F8E4, kind="ExternalInput")
    wdn_t = nc.dram_tensor("wdn_t", [P, IO * H], BF16, kind="ExternalInput")
    bup = nc.dram_tensor("bup", [P, IO], F32, kind="ExternalInput")
    bdn = nc.dram_tensor("bdn", [P, HO], F32, kind="ExternalInput")
    yt = nc.dram_tensor("yt", [H, CAP], BF16, kind="ExternalOutput")

    UPB = 4  # psum banks per tile-0 up-projection block
    DNB = 4  # psum banks per tile-0 down-projection block

    GELU_SCALE = 1.0 / SCALE
    T0 = TOKEN_TILES[0]

    with tile.TileContext(nc) as tc:
        with (
            tc.tile_pool(name="weights", bufs=1) as wpool,
            tc.tile_pool(name="xin", bufs=2) as xpool,
            tc.tile_pool(name="hmid", bufs=1) as hpool,
            tc.tile_pool(name="yout", bufs=4) as ypool,
            tc.tile_pool(name="psum_up", bufs=UPB, space="PSUM") as pu,
            tc.tile_pool(name="psum_dn", bufs=DNB, space="PSUM") as pd,
        ):
            yt_r = yt.ap().rearrange("(ho p) t -> p ho t", p=P)
            xt_ap = xt.ap()
            xt8_ap = xt8.ap()
            wup_ap = wup_t.ap()
            wup8_ap = wup8_t.ap()
            wdn_ap = wdn_t.ap()

            # DMA issue order is chosen so compute can start early: tile 0's
            # fp8 x rows + the first io-group's fp8 weights (0.37MB) gate the
            # first real matmul; each io group's remaining bf16 ko-chunks are
            # interleaved with x0's per-ko chunks so under the 8-core HBM
            # burst the per-step arrival cadence matches the ko-step compute.
            # The down weights stream in per-io chunks interleaved with tile
            # 0's up phase.
            UPG = 2 * UPB  # io tiles per tile-0 group
            x0_sb = xpool.tile([P, KB, T0], BF16, tag="x")
            x0_r = xt_ap[:, 0 : KB * T0].rearrange("p (ko t) -> p ko t", ko=KB)
            x80_sb = xpool.tile([P, 2, 448], F8E4, tag="x8")
            nc.sync.dma_start(
                x80_sb[:, :, :T0],
                xt8_ap[:, 0 : 2 * T0].rearrange("p (j t) -> p j t", j=2),
            )
            wup8_sb = wpool.tile([P, IO, 2, P], F8E4, tag="wup8")
            wup8_r = wup8_ap.rearrange("p (io j m) -> p io j m", io=IO, j=2)
            nc.sync.dma_start(wup8_sb[:, 0:UPG], wup8_r[:, 0:UPG])
            wup_sb = wpool.tile([P, KB, I], BF16, tag="wup")
            for ko in range(KB):
                nc.sync.dma_start(x0_sb[:, ko], x0_r[:, ko])
                nc.sync.dma_start(
                    wup_sb[:, ko, 0 : UPG * P],
                    wup_ap[:, ko * I : ko * I + UPG * P],
                )
            bup_sb = wpool.tile([P, IO], F32, tag="bup")
            nc.sync.dma_start(bup_sb[:], bup.ap())
            bdn_sb = wpool.tile([P, HO], F32, tag="bdn")
            nc.sync.dma_start(bdn_sb[:], bdn.ap())
            for iog in range(1, IO // UPG):
                nc.sync.dma_start(
                    wup8_sb[:, iog * UPG : (iog + 1) * UPG],
                    wup8_r[:, iog * UPG : (iog + 1) * UPG],
                )
                for ko in range(KB):
                    nc.sync.dma_start(
                        wup_sb[:, ko, iog * UPG * P : (iog + 1) * UPG * P],
                        wup_ap[:, ko * I + iog * UPG * P : ko * I + (iog + 1) * UPG * P],
                    )
            wdn_sb = wpool.tile([P, IO, H], BF16, tag="wdn")

            # Zeroed tile for warmup / keep-alive matmuls: they have no DMA
            # dependency, so the PE starts immediately and stays busy while
            # weights stream from HBM — keeping the HAM clock gate at full
            # rate.  They accumulate 0*0 = 0 into the first live psum group,
            # which is exact, so no extra psum bank is needed.
            xw_sb = wpool.tile([P, 512], BF16, tag="warmx")
            nc.vector.memset(xw_sb[:], 0.0)

            off = 0
            for t, ntok in enumerate(TOKEN_TILES):
                if t == 0:
                    x_sb = x0_sb
                    x8_sb = x80_sb
                else:
                    x_sb = xpool.tile([P, KB, T0], BF16, tag="x")
                    nc.sync.dma_start(
                        x_sb[:, :, :ntok],
                        xt_ap[:, KB * off : KB * (off + ntok)].rearrange(
                            "p (ko t) -> p ko t", ko=KB
                        ),
                    )
                    # free dim padded to 448 so the fp8 pair-dim stride is
                    # 16B-aligned as the DoubleRow AP requires.
                    x8_sb = xpool.tile([P, 2, 448], F8E4, tag="x8")
                    nc.sync.dma_start(
                        x8_sb[:, :, :ntok],
                        xt8_ap[:, 2 * off : 2 * (off + ntok)].rearrange(
                            "p (j t) -> p j t", j=2
                        ),
                    )

                # Up-projection + exact (erf) GELU: H^T tile [4096, ntok].
                h_sb = hpool.tile([P, IO, T0], BF16, tag="h")
                if t == 0:
                    # Tile 0: step-outer (DR, then ko 2..7) within a block of
                    # psum banks, so a block's matmuls can start as soon as
                    # the first weight chunk lands.  Blocks of 8 banks (the
                    # down pool is still idle) with keep-alive matmuls after
                    # each step of the first block: its pace is set by the
                    # up-weight DMA, and the fillers keep the HAM clock gate
                    # from re-throttling during the arrival gaps.
                    upb = 2 * UPB
                    for blk in range(IO // upb):
                        pss = [
                            (pu if j < UPB else pd).tile(
                                [P, T0], F32,
                                tag=("pu" if j < UPB else "pd"), name=f"pub{j}",
                            )
                            for j in range(upb)
                        ]
                        warm = blk == 0
                        if warm:
                            # PE warmup before the first data-dependent
                            # matmul: open pss[0]'s group with zeros, then
                            # bridge the PE to first-chunk arrival (~2-3us)
                            # while accumulating HAM busy time toward the
                            # 3.4us un-throttle window.
                            nc.tensor.matmul(
                                pss[0][:, :ntok], lhsT=xw_sb[:, :P],
                                rhs=xw_sb[:, :ntok], start=True, stop=False,
                            )
                            for _ in range(9):
                                nc.tensor.matmul(
                                    pss[0][:, :ntok], lhsT=xw_sb[:, :P],
                                    rhs=xw_sb[:, :ntok], start=False, stop=False,
                                )

                        def t0_step(step, j, blk=blk, pss=pss, warm=warm, ntok=ntok):
                            io = blk * upb + j
                            if step == 0:
                                nc.tensor.matmul(
                                    pss[j][:, :ntok],
                                    lhsT=wup8_sb[:, io],
                                    rhs=x80_sb[:, :, :ntok],
                                    start=not (warm and j == 0),
                                    stop=False,
                                    perf_mode=mybir.MatmulPerfMode.DoubleRow,
                                )
                            else:
                                nc.tensor.matmul(
                                    pss[j][:, :ntok],
                                    lhsT=wup_sb[:, step - 1, io * P : (io + 1) * P],
                                    rhs=x_sb[:, step - 1, :ntok],
                                    start=False,
                                    stop=(step == KB),
                                )

                        if blk == IO // upb - 1:
                            # Last tile-0 up block borrows the down pool's
                            # psum banks; close each accumulation group early
                            # (j-outer) so its GELU frees the bank while the
                            # rest of the block computes — otherwise the
                            # first down matmul stalls ~1.5us on the final
                            # four GELUs.
                            for j in range(upb):
                                for step in range(KB + 1):
                                    t0_step(step, j)
                        else:
                            for step in range(KB + 1):
                                for j in range(upb):
                                    t0_step(step, j)
                                if warm and step < KB:
                                    # Keep-alive against HBM-contention jitter.
                                    nc.tensor.matmul(
                                        pss[0][:, :ntok], lhsT=xw_sb[:, :P],
                                        rhs=xw_sb[:, :ntok], start=False, stop=False,
                                    )
                        for j in range(upb):
                            io = blk * upb + j
                            nc.scalar.activation(
                                h_sb[:, io, :ntok],
                                pss[j][:, :ntok],
                                mybir.ActivationFunctionType.Gelu,
                                bias=bup_sb[:, io : io + 1],
                                scale=GELU_SCALE,
                            )
                        # Stream the down weights while tile 0's up phase runs.
                        for io in range(blk * upb, (blk + 1) * upb):
                            nc.sync.dma_start(
                                wdn_sb[:, io], wdn_ap[:, io * H : (io + 1) * H]
                            )
                else:
                    # Tiles 1..4: weights fully resident, so run ko INNER in
                    # io PAIRS alternating between two psum banks — one fp8
                    # DoubleRow matmul plus six bf16 matmuls per io.
                    # Alternating banks avoids the ~2ns/matmul same-bank
                    # accumulation penalty, and each pair's GELUs hide under
                    # the next pair's 2.6us sweep with no group barrier.
                    for iop in range(IO // 2):
                        pspair = [
                            pu.tile([P, T0], F32, tag="pu", name=f"pus{j}")
                            for j in range(2)
                        ]
                        for j in range(2):
                            nc.tensor.matmul(
                                pspair[j][:, :ntok],
                                lhsT=wup8_sb[:, 2 * iop + j],
                                rhs=x8_sb[:, :, :ntok],
                                start=True,
                                stop=False,
                                perf_mode=mybir.MatmulPerfMode.DoubleRow,
                            )
                        for ko in range(KB):
                            for j in range(2):
                                io = 2 * iop + j
                                nc.tensor.matmul(
                                    pspair[j][:, :ntok],
                                    lhsT=wup_sb[:, ko, io * P : (io + 1) * P],
                                    rhs=x_sb[:, ko, :ntok],
                                    start=False,
                                    stop=(ko == KB - 1),
                                )
                        for j in range(2):
                            io = 2 * iop + j
                            nc.scalar.activation(
                                h_sb[:, io, :ntok],
                                pspair[j][:, :ntok],
                                mybir.ActivationFunctionType.Gelu,
                                bias=bup_sb[:, io : io + 1],
                                scale=GELU_SCALE,
                            )

                # Down-projection + bias: Y^T tile [1024, ntok] bf16 out.
                last = t == len(TOKEN_TILES) - 1
                if t == 0:
                    # Tile 0: contraction (io) outer within a DNB-bank block:
                    # the io demand is spread across the whole phase, matching
                    # the down-weight chunks still arriving from HBM.
                    for blk in range(HO // DNB):
                        ps2 = [
                            pd.tile([P, T0], F32, tag="pd", name=f"pd{j}")
                            for j in range(DNB)
                        ]
                        for io in range(IO):
                            for j in range(DNB):
                                ho = blk * DNB + j
                                nc.tensor.matmul(
                                    ps2[j][:, :ntok],
                                    lhsT=wdn_sb[:, io, ho * P : (ho + 1) * P],
                                    rhs=h_sb[:, io, :ntok],
                                    start=(io == 0),
                                    stop=(io == IO - 1),
                                )
                        for j in range(DNB):
                            ho = blk * DNB + j
                            y_sb = ypool.tile([P, T0], BF16, tag="y")
                            nc.vector.tensor_scalar_add(
                                y_sb[:, :ntok], ps2[j][:, :ntok], bdn_sb[:, ho : ho + 1]
                            )
                            nc.sync.dma_start(
                                yt_r[:, ho, off : off + ntok], y_sb[:, :ntok]
                            )
                else:
                    # Tiles 1..4: io-inner contraction sweeps over ho PAIRS
                    # alternating two psum banks: no same-bank accumulation
                    # penalty, and each pair's bias-add + output DMA overlaps
                    # the next pair's 11.8us sweep — no group barrier and a
                    # short serial kernel tail.
                    for hop in range(HO // 2):
                        pspair = [
                            pd.tile([P, T0], F32, tag="pd", name=f"pdl{j}")
                            for j in range(2)
                        ]
                        for io in range(IO):
                            for j in range(2):
                                ho = 2 * hop + j
                                nc.tensor.matmul(
                                    pspair[j][:, :ntok],
                                    lhsT=wdn_sb[:, io, ho * P : (ho + 1) * P],
                                    rhs=h_sb[:, io, :ntok],
                                    start=(io == 0),
                                    stop=(io == IO - 1),
                                )
                        for j in range(2):
                            ho = 2 * hop + j
                            y_sb = ypool.tile([P, T0], BF16, tag="y")
                            if last and ho == HO - 1:
                                # Split the very last bias+store so the final
                                # DMA starts half a tile earlier.
                                hn = ntok // 2
                                for sl in (slice(0, hn), slice(hn, ntok)):
                                    nc.vector.tensor_scalar_add(
                                        y_sb[:, sl], pspair[j][:, sl],
                                        bdn_sb[:, ho : ho + 1],
                                    )
                                    nc.sync.dma_start(
                                        yt_r[:, ho, off + sl.start : off + sl.stop],
                                        y_sb[:, sl],
                                    )
                            else:
                                nc.vector.tensor_scalar_add(
                                    y_sb[:, :ntok], pspair[j][:, :ntok],
                                    bdn_sb[:, ho : ho + 1],
                                )
                                nc.sync.dma_start(
                                    yt_r[:, ho, off : off + ntok], y_sb[:, :ntok]
                                )
                off += ntok

    nc.compile()
    return nc


def _get_program():
    global _compiled
    if _compiled is None:
        _compiled = _build_program()
    return _compiled


def _route(X64, Wg64):
    """Replicates the reference router: softmax over gate logits, top-2."""
    T = X64.shape[0]
    logits = X64 @ Wg64.T  # [T, E]
    logits -= logits.max(axis=-1, keepdims=True)
    p = np.exp(logits)
    p /= p.sum(axis=-1, keepdims=True)
    i1 = np.argmax(p, axis=-1)
    rows = np.arange(T)
    w1 = p[rows, i1]
    p2 = p.copy()
    p2[rows, i1] = -1.0
    i2 = np.argmax(p2, axis=-1)
    w2 = p[rows, i2]
    return i1, w1, i2, w2


def _q8(a):
    """Host e4m3 quantization (values pre-scaled), saturating."""
    return np.clip(a, -F8CLIP, F8CLIP).astype(ml_dtypes.float8_e4m3)


def kernel(hidden_states, Wg, Wup, bup, Wdown, bdown):
    global last_results
    hidden_states = np.asarray(hidden_states)
    orig_shape = hidden_states.shape
    X = np.ascontiguousarray(hidden_states, dtype=np.float32).reshape(-1, H)
    T = X.shape[0]
    Wg = np.asarray(Wg, dtype=np.float32)
    Wup = np.asarray(Wup, dtype=np.float32)
    bup = np.asarray(bup, dtype=np.float32)
    Wdown = np.asarray(Wdown, dtype=np.float32)
    bdown = np.asarray(bdown, dtype=np.float32)

    # --- Router on host (float64 for a faithful top-2 ordering) ---
    i1, w1, i2, w2 = _route(X.astype(np.float64), Wg.astype(np.float64))

    # --- Dispatch: gather each expert's tokens, pad to CAP ---
    KB = H // P - KO8
    Xb = X[:, KF8:].astype(ml_dtypes.bfloat16)  # bf16 ko-chunks 2..7 only
    in_maps = []
    meta = []
    for e in range(NUM_EXPERTS):
        sel1 = np.nonzero(i1 == e)[0]
        sel2 = np.nonzero(i2 == e)[0]
        idx = np.concatenate([sel1, sel2])
        wts = np.concatenate([w1[sel1], w2[sel2]])
        n = idx.size
        overflow = None
        if n > CAP:
            # Never expected for the reference inputs (max load 2161); kept as
            # a correctness safety net: spill tokens are computed on the host.
            overflow = (idx[CAP:], wts[CAP:])
            idx, wts = idx[:CAP], wts[:CAP]
            n = CAP
        idx_pad = np.concatenate([idx, np.zeros(CAP - n, dtype=idx.dtype)])
        # Pack into the kernel's DMA-native per-partition layouts:
        #  xt:  per token tile [P, KB, ntok] blocks (K-dims 256..1023, bf16)
        #  xt8: per token tile [P, 2, ntok] blocks (K-dims 0..255, fp8 *SX8)
        #  wup: Wup[:, 256:].T * SCALE as [P, KB*I]
        #  wup8: [P, IO, 2, 128] (fp8, *SW8);  wdn: Wdown.T as [P, IO*H]
        xt_full = Xb[idx_pad].T.reshape(KB, P, CAP)  # [KB, P, CAP]
        x8_full = _q8(X[idx_pad, :KF8] * SX8).reshape(CAP, 2, P)  # [CAP, 2, P]
        blocks = []
        blocks8 = []
        o = 0
        for ntok in TOKEN_TILES:
            blocks.append(xt_full[:, :, o : o + ntok].transpose(1, 0, 2).reshape(P, -1))
            blocks8.append(x8_full[o : o + ntok].transpose(2, 1, 0).reshape(P, -1))
            o += ntok
        xt_dev = np.concatenate(blocks, axis=1)
        xt8_dev = np.concatenate(blocks8, axis=1)
        wup_dev = (
            (Wup[e][:, KF8:] * SCALE).astype(ml_dtypes.bfloat16)
            .T.reshape(KB, P, I).transpose(1, 0, 2).reshape(P, -1)
        )
        # wup8[p, io, j, m] = Wup[io*128+m, 128*j+p] * SW8
        wup8_dev = (
            _q8(Wup[e][:, :KF8] * SW8)
            .reshape(I // P, P, 2, P)  # [io, m, j, p]
            .transpose(3, 0, 2, 1)
            .reshape(P, -1)
        )
        wdn_dev = (
            Wdown[e].astype(ml_dtypes.bfloat16).T.reshape(I // P, P, H)
            .transpose(1, 0, 2).reshape(P, -1)
        )
        in_maps.append(
            {
                "xt": np.ascontiguousarray(xt_dev),
                "xt8": np.ascontiguousarray(xt8_dev),
                "wup_t": np.ascontiguousarray(wup_dev),
                "wup8_t": np.ascontiguousarray(wup8_dev),
                "wdn_t": np.ascontiguousarray(wdn_dev),
                "bup": np.ascontiguousarray(bup[e].reshape(I // P, P).T),
                "bdn": np.ascontiguousarray(bdown[e].reshape(H // P, P).T),
            }
        )
        meta.append((idx, wts, overflow))

    # --- Run the Bass kernel on all 8 cores ---
    nc = _get_program()
    last_results = run_bass_kernel_spmd(nc, in_maps, core_ids=list(range(8)))

    # --- Combine: out[token] += w * y ---
    out = np.zeros((T, H), dtype=np.float32)
    for e in range(NUM_EXPERTS):
        idx, wts, overflow = meta[e]
        yt_full = np.asarray(last_results.results[e]["yt"])  # [H, CAP] bf16
        Y = yt_full.T[: idx.size].astype(np.float32)  # [n, H]
        out[idx] += wts[:, None].astype(np.float32) * Y
        if overflow is not None:
            oidx, owts = overflow
            from scipy.special import erf

            xo = X[oidx]
            h_in = xo @ Wup[e].T + bup[e]
            h = 0.5 * h_in * (1.0 + erf(h_in / np.sqrt(2.0)))
            yo = h @ Wdown[e].T + bdown[e]
            out[oidx] += owts[:, None].astype(np.float32) * yo
    return out.reshape(orig_shape)
